# revision 29
# baseline (speedup 1.0000x reference)
import math

import numpy as np

import jax
import jax.numpy as jnp

N = 2048
O = 64
CUTS = 8
RADIUS = 9.0
NL = 4
NC = 8
IPC = N // NC
EPS = 1e-5

_CACHE = {}


def _gn(x, weight, bias, groups=4):
    n, c = x.shape
    xt = x.T.reshape(groups, (c // groups) * n)
    mean = xt.mean(axis=1, keepdims=True)
    var = xt.var(axis=1, keepdims=True)
    xt = (xt - mean) * jax.lax.rsqrt(var + EPS)
    xt = xt.reshape(c, n)
    return (xt * weight[:, None] + bias[:, None]).T


def _lrelu(x):
    return jnp.where(x > 0, x, 0.2 * x)


def _build_fn(param_treedef):
    def body(dev, features, points, nuv, flat_params):
        # runs under pmap; axis 'i' over 8 devices; each device owns IPC rows
        params = jax.tree_util.tree_unflatten(param_treedef, flat_params)
        lo = dev * IPC
        pts = points / np.float32(math.sqrt(2.0) * RADIUS)
        normals = nuv[:, 0, :]
        pl = jax.lax.dynamic_slice_in_dim(pts, lo, IPC, 0)        # [IPC, 3]
        nuv_l = jax.lax.dynamic_slice_in_dim(nuv, lo, IPC, 0)     # [IPC, 3, 3]
        nl_ = nuv_l[:, 0, :]

        # pairwise window: layer-invariant
        diff = pts[None, :, :] - pl[:, None, :]                   # [IPC, N, 3]
        dot = nl_ @ normals.T                                     # [IPC, N]
        d2 = (diff ** 2).sum(-1) * (2.0 - dot) ** 2
        w = jnp.exp(-d2)                                          # [IPC, N]

        x = features
        for p in params:
            f = _lrelu(x @ p["in_w1"].T + p["in_b1"])
            f = _lrelu(f @ p["in_w2"].T + p["in_b2"])
            f = _gn(f, p["gn_in_w"], p["gn_in_b"])

            Xl = jnp.einsum("iac,ijc->ija", nuv_l, diff)          # [IPC, N, 3]
            h1 = jax.nn.relu(jnp.einsum("ija,ca->ijc", Xl, p["conv_w1"]) + p["conv_b1"])
            z = jnp.einsum("ijc,hc->ijh", h1, p["conv_w2"]) + p["conv_b2"]
            hloc = jnp.einsum("ij,ijh,jh->ih", w, jax.nn.relu(z), f)  # [IPC, O]

            h = jax.lax.all_gather(hloc, "i", axis=0).reshape(N, O)

            f2 = _lrelu(h @ p["out_w1"].T + p["out_b1"])
            f2 = _lrelu(f2 @ p["out_w2"].T + p["out_b2"])
            f2 = _gn(f2, p["gn_out_w"], p["gn_out_b"])
            hh = jax.nn.relu(f2 @ p["olay_w1"].T + p["olay_b1"])
            hh = hh @ p["olay_w2"].T + p["olay_b2"]
            x = x @ p["ilay_w"].T + p["ilay_b"] + hh
        return x

    return jax.pmap(body, axis_name="i",
                    in_axes=(0, None, None, None, None),
                    out_axes=None,
                    devices=jax.devices()[:NC])


def kernel(features, points, nuv, ranges, params):
    features = jnp.asarray(np.asarray(features, np.float32))
    points = jnp.asarray(np.asarray(points, np.float32))
    nuv = jnp.asarray(np.asarray(nuv, np.float32))
    params_np = [{k: jnp.asarray(np.asarray(v, np.float32)) for k, v in p.items()}
                 for p in params]
    flat, treedef = jax.tree_util.tree_flatten(params_np)
    key = "fn"
    if key not in _CACHE:
        _CACHE[key] = _build_fn(treedef)
    dev_ids = jnp.arange(NC, dtype=jnp.int32)
    out = _CACHE[key](dev_ids, features, points, nuv, flat)
    return np.asarray(jax.device_get(out), np.float32)


# revision 30
# speedup vs baseline: 1.0709x; 1.0709x over previous
import math

import numpy as np

import jax
import jax.numpy as jnp

N = 2048
O = 64
CUTS = 8
RADIUS = 9.0
NL = 4
NC = 8
IPC = N // NC
EPS = 1e-5

_CACHE = {}


def _gn(x, weight, bias, groups=4):
    n, c = x.shape
    xt = x.T.reshape(groups, (c // groups) * n)
    mean = xt.mean(axis=1, keepdims=True)
    var = xt.var(axis=1, keepdims=True)
    xt = (xt - mean) * jax.lax.rsqrt(var + EPS)
    xt = xt.reshape(c, n)
    return (xt * weight[:, None] + bias[:, None]).T


def _lrelu(x):
    return jnp.where(x > 0, x, 0.2 * x)


def _build_fn(param_treedef):
    def body(dev, features, points, nuv, flat_params):
        # runs under pmap; axis 'i' over 8 devices; each device owns IPC rows
        params = jax.tree_util.tree_unflatten(param_treedef, flat_params)
        lo = dev * IPC
        pts = points / np.float32(math.sqrt(2.0) * RADIUS)
        normals = nuv[:, 0, :]
        pl = jax.lax.dynamic_slice_in_dim(pts, lo, IPC, 0)        # [IPC, 3]
        nuv_l = jax.lax.dynamic_slice_in_dim(nuv, lo, IPC, 0)     # [IPC, 3, 3]
        nl_ = nuv_l[:, 0, :]

        # pairwise window: layer-invariant
        diff = pts[None, :, :] - pl[:, None, :]                   # [IPC, N, 3]
        dot = nl_ @ normals.T                                     # [IPC, N]
        d2 = (diff ** 2).sum(-1) * (2.0 - dot) ** 2
        w = jnp.exp(-d2)                                          # [IPC, N]

        Xl = jnp.einsum("iac,ijc->ija", nuv_l, diff)              # [IPC, N, 3]
        w16 = w[:, :, None].astype(jnp.bfloat16)                  # [IPC, N, 1]

        x = features
        for p in params:
            f = _lrelu(x @ p["in_w1"].T + p["in_b1"])
            f = _lrelu(f @ p["in_w2"].T + p["in_b2"])
            f = _gn(f, p["gn_in_w"], p["gn_in_b"])
            f16 = f.astype(jnp.bfloat16)

            h1 = jax.nn.relu(jnp.einsum("ija,ca->ijc", Xl, p["conv_w1"]) + p["conv_b1"])
            # fold the (positive) window and the MLP2 bias into a 9-channel
            # bf16 matmul: relu(w*z) = w*relu(z), bias enters via the w channel
            h1w9 = jnp.concatenate(
                [h1.astype(jnp.bfloat16) * w16, w16], axis=2)     # [IPC, N, 9]
            w2e = jnp.concatenate(
                [p["conv_w2"], p["conv_b2"][:, None]], axis=1).astype(jnp.bfloat16)
            zw = jnp.einsum("ijc,hc->ijh", h1w9, w2e,
                            preferred_element_type=jnp.float32)   # [IPC, N, 64]
            rz = jax.nn.relu(zw).astype(jnp.bfloat16)
            hloc = jnp.einsum("ijh,jh->ih", rz, f16,
                              preferred_element_type=jnp.float32)  # [IPC, O]

            h = jax.lax.all_gather(hloc, "i", axis=0).reshape(N, O)

            f2 = _lrelu(h @ p["out_w1"].T + p["out_b1"])
            f2 = _lrelu(f2 @ p["out_w2"].T + p["out_b2"])
            f2 = _gn(f2, p["gn_out_w"], p["gn_out_b"])
            hh = jax.nn.relu(f2 @ p["olay_w1"].T + p["olay_b1"])
            hh = hh @ p["olay_w2"].T + p["olay_b2"]
            x = x @ p["ilay_w"].T + p["ilay_b"] + hh
        return x

    return jax.pmap(body, axis_name="i",
                    in_axes=(0, None, None, None, None),
                    out_axes=None,
                    devices=jax.devices()[:NC])


def kernel(features, points, nuv, ranges, params):
    features = jnp.asarray(np.asarray(features, np.float32))
    points = jnp.asarray(np.asarray(points, np.float32))
    nuv = jnp.asarray(np.asarray(nuv, np.float32))
    params_np = [{k: jnp.asarray(np.asarray(v, np.float32)) for k, v in p.items()}
                 for p in params]
    flat, treedef = jax.tree_util.tree_flatten(params_np)
    key = "fn"
    if key not in _CACHE:
        _CACHE[key] = _build_fn(treedef)
    dev_ids = jnp.arange(NC, dtype=jnp.int32)
    out = _CACHE[key](dev_ids, features, points, nuv, flat)
    return np.asarray(jax.device_get(out), np.float32)
